# revision 7
# baseline (speedup 1.0000x reference)
"""Trainium2 Bass kernel for nn_CompetitiveNetwork (competitive-binding solve).

Math (per batch column):
    K  = clip(exp(K_raw), 0, 1e3)   BT = clip(exp(BT_raw), 0, 1e3)
    fixed point:  u = 1/(1 + K^T AF);  AF = AT / (1 + (K diag(BT)) u)
    readout:      Y = sum_b (M^T AF)_b * u_b + b,  M = K*W*BT

Device algorithm (accelerated, validated numerically on host):
  state G (gain; AF = AT*G) and u. Per iteration:
    S  = W1blk @ af          (fp32r matmul, 128-part block-diag = 2 streams)
    u  = 1/(1+S)             (ACT reciprocal, exact)
    Tw = wg*W2blk @ u        (fp32r matmul; wg folded into weights)
    G  = ((1+wg) - (Tw+wg))*G)*G   one fused DVE op == SOR(wg) + Newton
         (iters 1..E_G use the exact path: ACT recip + scalar_tensor_tensor)
    af = AT*G                (DVE/Pool mul)
  9 over-relaxed iterations (wg=1.4) replace the reference's 21.5 plain
  iterations: SOR contraction ~0.33/iter vs 0.6, landing ~1.8e-3 from the
  reference iterate (fixed-point limit itself is only ~9e-5 away).
  All matmuls use fp32r (1 cyc/row at FD>=256; measured 9e-5 accurate on hw).

Sharding: pure data-parallel over batch (16384 -> 8 cores x 2048).
Layout: features on partitions, batch on free dim; two 64-partition streams
stacked into (128, FD) tiles; 4 column chunks of FD=256.
"""

import numpy as np

import concourse.bacc as bacc
import concourse.mybir as mybir
from concourse.tile import TileContext
from concourse.bass_utils import run_bass_kernel_spmd


# --- custom DVE op: NEWTON1P_ANT ---
# out = (c1 - (in0 + c0) * in1) * in1 : with (c0,c1)=(1,2) one Newton step
# of in1 toward 1/(1+in0); with (c0,c1)=(w,1+w) and in0=w*T it fuses the
# Newton step with SOR mixing: out = (1-w)*in1 + w*newton(in1; 1+T).

import concourse.dve_ops as dve_ops
from concourse.dve_ops import DveOp
from concourse.dve_spec import Spec, Src0, Src1, C0, C1, lower


def _ref_newton1p(in0, in1, c0, c1, c2):
    return ((c1 - (in0.astype(np.float32) + c0) * in1) * in1).astype(np.float32)


def _make_op(shas):
    return DveOp(
        "NEWTON1P_ANT",
        Spec(
            body=(C1 - (Src0 + C0) * Src1) * Src1,
            reference=_ref_newton1p,
        ),
        subdim=False,
        uops_sha=shas,
    )


def register():
    for op in dve_ops.OPS:
        if op.name == "NEWTON1P_ANT":
            return op
    probe = _make_op({})
    opcode = dve_ops._CUSTOM_DVE_ROW_BASE + len(dve_ops.OPS)
    shas = {}
    for ver in ("v3", "v4"):
        try:
            from concourse.dve_uop import DveOpSpec
            res = DveOpSpec(name=probe.name, opcode=opcode,
                            uops=lower(probe.spec, ver=ver),
                            rd1_en=True)
            shas[ver] = res.sha(ver)
        except Exception as e:
            print(f"lower {ver} failed: {e}")
    op = _make_op(shas)
    dve_ops.OPS.append(op)
    dve_ops.CUSTOM_DVE_SPECS[op.name] = op.spec
    dve_ops._SUB_OPCODE_FOR_NAME[op.name] = (
        dve_ops._CUSTOM_DVE_ROW_BASE + len(dve_ops.OPS) - 1)
    return op


def newton_sor(nc_vector, out, in0, in1, c0, c1):
    """out = (c1 - (in0 + c0) * in1) * in1 on the DVE."""
    op = register()
    return nc_vector._custom_dve(op, out=out, in0=in0, in1=in1,
                                 s0=float(c0), s1=float(c1), imm2=0.0)


B, NA, NB = 16384, 64, 64
N_CORES = 8
B_CORE = B // N_CORES          # 2048 batch columns per core
N_CHUNK = 4
FD = B_CORE // 2 // N_CHUNK    # 256

N_ITERS = 9                    # over-relaxed iterations
WG = 1.4                       # SOR factor on the G (gain) update
E_G = 3                        # iters 1..E_G use the exact ACT+stt G path

FP32 = mybir.dt.float32
FP32R = mybir.dt.float32r

_CACHE = {}


def _act_recip(nc, out_ap, in_ap, bias=1.0, scale=1.0):
    """out = 1/(in*scale + bias) on the Activation engine."""
    eng = nc.scalar
    ins = [eng.lower_ap(in_ap),
           mybir.ImmediateValue(dtype=FP32, value=float(bias)),
           mybir.ImmediateValue(dtype=FP32, value=float(scale)),
           mybir.ImmediateValue(dtype=FP32, value=0.0)]
    eng.add_instruction(mybir.InstActivation(
        name=nc.get_next_instruction_name(),
        func=mybir.ActivationFunctionType.Reciprocal,
        ins=ins, outs=[eng.lower_ap(out_ap)]))


def _build_module(repeat=1):
    register()
    nc = bacc.Bacc()
    att = nc.dram_tensor("att", (128, N_CHUNK * FD), FP32, kind="ExternalInput")
    # weights: [W1blk | W2a | W2b | M2blk | ones2]  (128, 514)
    wts = nc.dram_tensor("wts", (128, 4 * 128 + 2), FP32, kind="ExternalInput")
    yout = nc.dram_tensor("yout", (2 * N_CHUNK, FD), FP32, kind="ExternalOutput")

    with TileContext(nc) as tc, \
         tc.tile_pool(name="const", bufs=1) as cpool, \
         tc.tile_pool(name="state", bufs=2) as spool, \
         tc.tile_pool(name="work", bufs=3) as wpool, \
         tc.tile_pool(name="psum", bufs=8, space="PSUM") as ppool:

        wall = cpool.tile([128, 4 * 128 + 2], FP32, tag="wall")
        nc.sync.dma_start(out=wall[:], in_=wts[:, :])
        wallr = cpool.tile([128, 4 * 128 + 2], FP32R, tag="wallr")
        nc.vector.tensor_copy(wallr[:], wall[:])
        w1r = wallr[:, 0:128]
        w2ar = wallr[:, 128:256]
        w2br = wallr[:, 256:384]
        m2r = wallr[:, 384:512]
        onesr = wallr[:, 512:514]

        ats, atrs = [], []
        for c in range(N_CHUNK):
            at_c = cpool.tile([128, FD], FP32, tag=f"at{c}")
            nc.sync.dma_start(out=at_c[:], in_=att[:, c * FD:(c + 1) * FD])
            ats.append(at_c)
            atr_c = cpool.tile([128, FD], FP32R, tag=f"atr{c}")
            nc.vector.tensor_copy(atr_c[:], at_c[:])
            atrs.append(atr_c)

        for _rep in range(repeat):
            af = list(atrs)             # AF_0 = AT (G_0 = 1)
            us = [None] * N_CHUNK
            gs = [None] * N_CHUNK
            banks = [None] * N_CHUNK

            def emit_halfstep(c, h):
                n = h // 2 + 1          # iteration number, 1-based
                if h % 2 == 0:
                    # S-side: ps = W1 af ; u = 1/(1+S) on ACT
                    # one PSUM bank per chunk-iteration: mm1 -> half 0,
                    # mm2 -> half 1 (PSUM tiles are bank-granular)
                    bank = ppool.tile([128, 2 * FD], FP32, tag="ps")
                    banks[c] = bank
                    ps = bank[:, 0:FD]
                    nc.tensor.matmul(out=ps, lhsT=w1r, rhs=af[c][:],
                                     start=True, stop=True)
                    u_n = spool.tile([128, FD], FP32R, tag=f"u{c}")
                    _act_recip(nc, u_n[:], ps)
                    us[c] = u_n
                else:
                    # T-side: ps2 = wg*W2 u ; G update ; af = AT*G
                    ps2 = banks[c][:, FD:2 * FD]
                    nc.tensor.matmul(out=ps2, lhsT=(w2ar if n == 1 else w2br),
                                     rhs=us[c][:], start=True, stop=True)
                    g_n = spool.tile([128, FD], FP32, tag=f"g{c}")
                    if n == 1:
                        # G_1 = 1/(1+T)   (wg=1 first step)
                        _act_recip(nc, g_n[:], ps2)
                    elif n <= E_G:
                        # exact SOR: ACT emits R = wg/(1+T) directly
                        # (1/(ps2*s + b) with s=1/wg^2, b=1/wg on ps2=wg*T),
                        # then G = (1-wg)*G + R in one stt.
                        r_n = wpool.tile([128, FD], FP32, tag=f"r{c}")
                        _act_recip(nc, r_n[:], ps2,
                                   bias=1.0 / WG, scale=1.0 / (WG * WG))
                        nc.vector.scalar_tensor_tensor(
                            out=g_n[:], in0=gs[c][:], scalar=float(1.0 - WG),
                            in1=r_n[:], op0=mybir.AluOpType.mult,
                            op1=mybir.AluOpType.add)
                    else:
                        # fused Newton+SOR: G = ((1+wg)-(ps2+wg)*G)*G
                        newton_sor(nc.vector, g_n[:], ps2, gs[c][:],
                                   WG, 1.0 + WG)
                    gs[c] = g_n
                    af_n = spool.tile([128, FD], FP32R, tag=f"af{c}")
                    if c < 2:
                        nc.vector.tensor_mul(af_n[:], ats[c][:], gs[c][:])
                    else:
                        nc.gpsimd.tensor_mul(af_n[:], ats[c][:], gs[c][:])
                    af[c] = af_n

            # group B (chunks 2,3) runs one half-step behind group A (0,1):
            # each tick mixes S-side and T-side work so every engine's
            # in-order stream has a steady supply of ready instructions.
            H = 2 * N_ITERS
            for t in range(H + 1):
                for c in (0, 1):
                    if t < H:
                        emit_halfstep(c, t)
                for c in (2, 3):
                    if t >= 1:
                        emit_halfstep(c, t - 1)

            # readout: S = W1 af; u = newton(u); V = M2 af; h = V*u; Y = ones^T h
            for c in range(N_CHUNK):
                bank = ppool.tile([128, 2 * FD], FP32, tag="ps")
                ps = bank[:, 0:FD]
                pp = bank[:, FD:2 * FD]
                nc.tensor.matmul(out=ps, lhsT=w1r, rhs=af[c][:],
                                 start=True, stop=True)
                nc.tensor.matmul(out=pp, lhsT=m2r, rhs=af[c][:],
                                 start=True, stop=True)
                u1 = spool.tile([128, FD], FP32, tag=f"u{c}")
                newton_sor(nc.vector, u1[:], ps, us[c][:], 1.0, 2.0)
                u2 = spool.tile([128, FD], FP32, tag=f"uf{c}")
                newton_sor(nc.vector, u2[:], ps, u1[:], 1.0, 2.0)
                h = wpool.tile([128, FD], FP32R, tag="h")
                nc.vector.tensor_mul(h[:], pp, u2[:])
                ybank = ppool.tile([128, 2 * FD], FP32, tag="ps")
                yp = ybank[:, 0:FD]
                nc.tensor.matmul(out=yp[0:2, :], lhsT=onesr,
                                 rhs=h[:], start=True, stop=True)
                ys = wpool.tile([128, FD], FP32, tag="ys")
                nc.scalar.copy(ys[0:2, :], yp[0:2, :])
                nc.sync.dma_start(out=yout[2 * c:2 * c + 2, :], in_=ys[0:2, :])

    nc.finalize()
    return nc


def _get_module(repeat=1):
    key = f"nc{repeat}"
    if key not in _CACHE:
        _CACHE[key] = _build_module(repeat)
    return _CACHE[key]


def kernel(AT, K_raw, BT_raw, W_raw, b_raw, _run_kw=None, _repeat=1):
    AT = np.asarray(AT, dtype=np.float32)
    K = np.clip(np.exp(np.asarray(K_raw, np.float32)), 0.0, 1000.0).astype(np.float32)
    BT = np.clip(np.exp(np.asarray(BT_raw, np.float32)), 0.0, 1000.0).astype(np.float32)
    Wc = np.clip(np.asarray(W_raw, np.float32), -10.0, 10.0).reshape(NA, NB)
    b0 = np.clip(np.asarray(b_raw, np.float32), -10.0, 10.0)[0]

    w2 = np.ascontiguousarray((K * BT[None, :]).T)     # (nB,nA) lhsT: T = w2^T u
    M = K * Wc * BT[None, :]                           # (nA,nB) lhsT: V = M^T af

    def blk(a):
        z = np.zeros((128, 128), np.float32)
        z[0:64, 0:64] = a
        z[64:128, 64:128] = a
        return z

    ones2 = np.zeros((128, 2), np.float32)
    ones2[0:64, 0] = 1.0
    ones2[64:128, 1] = 1.0
    wts = np.ascontiguousarray(np.concatenate(
        [blk(K), blk(w2), blk(WG * w2), blk(M), ones2], axis=1))

    att = np.ascontiguousarray(AT.T)                   # (64, 16384)
    HB = B_CORE // 2                                   # 1024 cols per stream
    in_maps = []
    for c in range(N_CORES):
        chunk = att[:, c * B_CORE:(c + 1) * B_CORE]    # (64, 2048)
        stacked = np.ascontiguousarray(
            np.concatenate([chunk[:, :HB], chunk[:, HB:]], axis=0))
        in_maps.append({"att": stacked, "wts": wts})

    nc = _get_module(_repeat)
    res = run_bass_kernel_spmd(nc, in_maps, core_ids=list(range(N_CORES)),
                               **(_run_kw or {}))
    out = np.empty((B,), np.float32)
    for co in range(N_CORES):
        y = res.results[co]["yout"]                    # (8, 256)
        base = co * B_CORE
        for c in range(N_CHUNK):
            out[base + c * FD:base + (c + 1) * FD] = y[2 * c]
            out[base + HB + c * FD:base + HB + (c + 1) * FD] = y[2 * c + 1]
    if _run_kw is not None:
        _CACHE["last_result"] = res
    return out + b0


# revision 10
# speedup vs baseline: 1.0118x; 1.0118x over previous
"""Trainium2 Bass kernel for nn_CompetitiveNetwork (competitive-binding solve).

Math (per batch column):
    K  = clip(exp(K_raw), 0, 1e3)   BT = clip(exp(BT_raw), 0, 1e3)
    fixed point:  u = 1/(1 + K^T AF);  AF = AT / (1 + (K diag(BT)) u)
    readout:      Y = sum_b (M^T AF)_b * u_b + b,  M = K*W*BT

Device algorithm (accelerated, validated numerically on host):
  state G (gain; AF = AT*G) and u. Per iteration:
    S  = W1blk @ af          (fp32r matmul, 128-part block-diag = 2 streams)
    u  = 1/(1+S)             (ACT reciprocal, exact)
    Tw = wg*W2blk @ u        (fp32r matmul; wg folded into weights)
    G  = ((1+wg) - (Tw+wg))*G)*G   one fused DVE op == SOR(wg) + Newton
         (iters 1..E_G use the exact path: ACT recip + scalar_tensor_tensor)
    af = AT*G                (DVE/Pool mul)
  9 over-relaxed iterations (wg=1.4) replace the reference's 21.5 plain
  iterations: SOR contraction ~0.33/iter vs 0.6, landing ~1.8e-3 from the
  reference iterate (fixed-point limit itself is only ~9e-5 away).
  All matmuls use fp32r (1 cyc/row at FD>=256; measured 9e-5 accurate on hw).

Sharding: pure data-parallel over batch (16384 -> 8 cores x 2048).
Layout: features on partitions, batch on free dim; two 64-partition streams
stacked into (128, FD) tiles; 4 column chunks of FD=256.
"""

import numpy as np

import concourse.bacc as bacc
import concourse.mybir as mybir
from concourse.tile import TileContext
from concourse.bass_utils import run_bass_kernel_spmd


# --- custom DVE op: NEWTON1P_ANT ---
# out = (c1 - (in0 + c0) * in1) * in1 : with (c0,c1)=(1,2) one Newton step
# of in1 toward 1/(1+in0); with (c0,c1)=(w,1+w) and in0=w*T it fuses the
# Newton step with SOR mixing: out = (1-w)*in1 + w*newton(in1; 1+T).

import concourse.dve_ops as dve_ops
from concourse.dve_ops import DveOp
from concourse.dve_spec import Spec, Src0, Src1, C0, C1, lower


def _ref_newton1p(in0, in1, c0, c1, c2):
    return ((c1 - (in0.astype(np.float32) + c0) * in1) * in1).astype(np.float32)


def _make_op(shas):
    return DveOp(
        "NEWTON1P_ANT",
        Spec(
            body=(C1 - (Src0 + C0) * Src1) * Src1,
            reference=_ref_newton1p,
        ),
        subdim=False,
        uops_sha=shas,
    )


def register():
    for op in dve_ops.OPS:
        if op.name == "NEWTON1P_ANT":
            return op
    probe = _make_op({})
    opcode = dve_ops._CUSTOM_DVE_ROW_BASE + len(dve_ops.OPS)
    shas = {}
    for ver in ("v3", "v4"):
        try:
            from concourse.dve_uop import DveOpSpec
            res = DveOpSpec(name=probe.name, opcode=opcode,
                            uops=lower(probe.spec, ver=ver),
                            rd1_en=True)
            shas[ver] = res.sha(ver)
        except Exception as e:
            print(f"lower {ver} failed: {e}")
    op = _make_op(shas)
    dve_ops.OPS.append(op)
    dve_ops.CUSTOM_DVE_SPECS[op.name] = op.spec
    dve_ops._SUB_OPCODE_FOR_NAME[op.name] = (
        dve_ops._CUSTOM_DVE_ROW_BASE + len(dve_ops.OPS) - 1)
    return op


def newton_sor(nc_vector, out, in0, in1, c0, c1):
    """out = (c1 - (in0 + c0) * in1) * in1 on the DVE."""
    op = register()
    return nc_vector._custom_dve(op, out=out, in0=in0, in1=in1,
                                 s0=float(c0), s1=float(c1), imm2=0.0)


B, NA, NB = 16384, 64, 64
N_CORES = 8
B_CORE = B // N_CORES          # 2048 batch columns per core
N_CHUNK = 4
FD = B_CORE // 2 // N_CHUNK    # 256

N_ITERS = 9                    # over-relaxed iterations
WG = 1.4                       # SOR factor on the G (gain) update
E_G = 3                        # iters 1..E_G use the exact ACT+stt G path

FP32 = mybir.dt.float32
FP32R = mybir.dt.float32r

_CACHE = {}


def _act_recip(nc, out_ap, in_ap, bias=1.0, scale=1.0):
    """out = 1/(in*scale + bias) on the Activation engine."""
    eng = nc.scalar
    ins = [eng.lower_ap(in_ap),
           mybir.ImmediateValue(dtype=FP32, value=float(bias)),
           mybir.ImmediateValue(dtype=FP32, value=float(scale)),
           mybir.ImmediateValue(dtype=FP32, value=0.0)]
    eng.add_instruction(mybir.InstActivation(
        name=nc.get_next_instruction_name(),
        func=mybir.ActivationFunctionType.Reciprocal,
        ins=ins, outs=[eng.lower_ap(out_ap)]))


def _build_module(repeat=1):
    register()
    nc = bacc.Bacc()
    att = nc.dram_tensor("att", (128, N_CHUNK * FD), FP32, kind="ExternalInput")
    # weights: [W1blk | W2a | W2b | M2blk | ones2]  (128, 514)
    wts = nc.dram_tensor("wts", (128, 4 * 128 + 2), FP32, kind="ExternalInput")
    yout = nc.dram_tensor("yout", (2, N_CHUNK * FD), FP32, kind="ExternalOutput")

    with TileContext(nc) as tc, \
         tc.tile_pool(name="const", bufs=1) as cpool, \
         tc.tile_pool(name="state", bufs=2) as spool, \
         tc.tile_pool(name="work", bufs=3) as wpool, \
         tc.tile_pool(name="psum", bufs=8, space="PSUM") as ppool:

        wall = cpool.tile([128, 4 * 128 + 2], FP32, tag="wall")
        nc.sync.dma_start(out=wall[:], in_=wts[:, :])
        wallr = cpool.tile([128, 4 * 128 + 2], FP32R, tag="wallr")
        nc.vector.tensor_copy(wallr[:], wall[:])
        w1f = wall[:, 0:128]           # fp32 W1 for the first iteration
        w1r = wallr[:, 0:128]
        w2ar = wallr[:, 128:256]
        w2br = wallr[:, 256:384]
        m2r = wallr[:, 384:512]
        onesr = wallr[:, 512:514]

        # spread the input DMA issues across engine queues (parallel issue)
        ats = []
        dma_eng = [nc.scalar, nc.gpsimd, nc.scalar, nc.gpsimd]
        for c in range(N_CHUNK):
            at_c = cpool.tile([128, FD], FP32, tag=f"at{c}")
            dma_eng[c].dma_start(out=at_c[:], in_=att[:, c * FD:(c + 1) * FD])
            ats.append(at_c)

        for _rep in range(repeat):
            af = list(ats)              # AF_0 = AT (G_0 = 1; fp32, used once)
            us = [None] * N_CHUNK
            gs = [None] * N_CHUNK
            banks = [None] * N_CHUNK

            def emit_halfstep(c, h):
                n = h // 2 + 1          # iteration number, 1-based
                if h % 2 == 0:
                    # S-side: ps = W1 af ; u = 1/(1+S) on ACT
                    # one PSUM bank per chunk-iteration: mm1 -> half 0,
                    # mm2 -> half 1 (PSUM tiles are bank-granular)
                    bank = ppool.tile([128, 2 * FD], FP32, tag="ps")
                    banks[c] = bank
                    ps = bank[:, 0:FD]
                    # iteration 1 runs in fp32 (af = AT straight from DMA);
                    # later iterations in fp32r (1 cyc/row)
                    nc.tensor.matmul(out=ps, lhsT=(w1f if n == 1 else w1r),
                                     rhs=af[c][:], start=True, stop=True)
                    u_n = spool.tile([128, FD], FP32R, tag=f"u{c}")
                    _act_recip(nc, u_n[:], ps)
                    us[c] = u_n
                else:
                    # T-side: ps2 = wg*W2 u ; G update ; af = AT*G
                    ps2 = banks[c][:, FD:2 * FD]
                    nc.tensor.matmul(out=ps2, lhsT=(w2ar if n == 1 else w2br),
                                     rhs=us[c][:], start=True, stop=True)
                    g_n = spool.tile([128, FD], FP32, tag=f"g{c}")
                    if n == 1:
                        # G_1 = 1/(1+T)   (wg=1 first step)
                        _act_recip(nc, g_n[:], ps2)
                    elif n <= E_G:
                        # exact SOR: ACT emits R = wg/(1+T) directly
                        # (1/(ps2*s + b) with s=1/wg^2, b=1/wg on ps2=wg*T),
                        # then G = (1-wg)*G + R in one stt.
                        r_n = wpool.tile([128, FD], FP32, tag=f"r{c}")
                        _act_recip(nc, r_n[:], ps2,
                                   bias=1.0 / WG, scale=1.0 / (WG * WG))
                        nc.vector.scalar_tensor_tensor(
                            out=g_n[:], in0=gs[c][:], scalar=float(1.0 - WG),
                            in1=r_n[:], op0=mybir.AluOpType.mult,
                            op1=mybir.AluOpType.add)
                    else:
                        # fused Newton+SOR: G = ((1+wg)-(ps2+wg)*G)*G
                        newton_sor(nc.vector, g_n[:], ps2, gs[c][:],
                                   WG, 1.0 + WG)
                    gs[c] = g_n
                    af_n = spool.tile([128, FD], FP32R, tag=f"af{c}")
                    if c < 2:
                        nc.vector.tensor_mul(af_n[:], ats[c][:], gs[c][:])
                    else:
                        nc.gpsimd.tensor_mul(af_n[:], ats[c][:], gs[c][:])
                    af[c] = af_n

            # chunk c runs c half-steps behind chunk 0: four phases in
            # flight so every engine queue always holds ready work.
            H = 2 * N_ITERS
            for t in range(H + N_CHUNK):
                for c in range(N_CHUNK):
                    h = t - c
                    if 0 <= h < H:
                        emit_halfstep(c, h)

            # readout: S = W1 af; u = newton(u); V = M2 af; h = V*u;
            # Y = ones^T h. h packed pairwise -> one ones-matmul per pair.
            hpair0 = wpool.tile([128, 2 * FD], FP32R, tag="hp0")
            hpair1 = wpool.tile([128, 2 * FD], FP32R, tag="hp1")
            hpair = [hpair0, hpair1]
            for c in range(N_CHUNK):
                bank = ppool.tile([128, 2 * FD], FP32, tag="ps")
                ps = bank[:, 0:FD]
                pp = bank[:, FD:2 * FD]
                nc.tensor.matmul(out=ps, lhsT=w1r, rhs=af[c][:],
                                 start=True, stop=True)
                nc.tensor.matmul(out=pp, lhsT=m2r, rhs=af[c][:],
                                 start=True, stop=True)
                u1 = spool.tile([128, FD], FP32, tag=f"u{c}")
                newton_sor(nc.vector, u1[:], ps, us[c][:], 1.0, 2.0)
                hslot = hpair[c // 2][:, (c % 2) * FD:(c % 2 + 1) * FD]
                nc.vector.tensor_mul(hslot, pp, u1[:])
                if c % 2 == 1:
                    p = c // 2
                    ybank = ppool.tile([128, 2 * FD], FP32, tag="ps")
                    nc.tensor.matmul(out=ybank[0:2, :], lhsT=onesr,
                                     rhs=hpair[p][:], start=True, stop=True)
                    ys = wpool.tile([128, 2 * FD], FP32, tag=f"ys{p}")
                    nc.vector.tensor_copy(ys[0:2, :], ybank[0:2, :])
                    nc.sync.dma_start(out=yout[:, p * 2 * FD:(p + 1) * 2 * FD],
                                      in_=ys[0:2, :])

    nc.finalize()
    return nc


def _get_module(repeat=1):
    key = f"nc{repeat}"
    if key not in _CACHE:
        _CACHE[key] = _build_module(repeat)
    return _CACHE[key]


def kernel(AT, K_raw, BT_raw, W_raw, b_raw, _run_kw=None, _repeat=1):
    AT = np.asarray(AT, dtype=np.float32)
    K = np.clip(np.exp(np.asarray(K_raw, np.float32)), 0.0, 1000.0).astype(np.float32)
    BT = np.clip(np.exp(np.asarray(BT_raw, np.float32)), 0.0, 1000.0).astype(np.float32)
    Wc = np.clip(np.asarray(W_raw, np.float32), -10.0, 10.0).reshape(NA, NB)
    b0 = np.clip(np.asarray(b_raw, np.float32), -10.0, 10.0)[0]

    w2 = np.ascontiguousarray((K * BT[None, :]).T)     # (nB,nA) lhsT: T = w2^T u
    M = K * Wc * BT[None, :]                           # (nA,nB) lhsT: V = M^T af

    def blk(a):
        z = np.zeros((128, 128), np.float32)
        z[0:64, 0:64] = a
        z[64:128, 64:128] = a
        return z

    ones2 = np.zeros((128, 2), np.float32)
    ones2[0:64, 0] = 1.0
    ones2[64:128, 1] = 1.0
    wts = np.ascontiguousarray(np.concatenate(
        [blk(K), blk(w2), blk(WG * w2), blk(M), ones2], axis=1))

    att = np.ascontiguousarray(AT.T)                   # (64, 16384)
    HB = B_CORE // 2                                   # 1024 cols per stream
    in_maps = []
    for c in range(N_CORES):
        chunk = att[:, c * B_CORE:(c + 1) * B_CORE]    # (64, 2048)
        stacked = np.ascontiguousarray(
            np.concatenate([chunk[:, :HB], chunk[:, HB:]], axis=0))
        in_maps.append({"att": stacked, "wts": wts})

    nc = _get_module(_repeat)
    res = run_bass_kernel_spmd(nc, in_maps, core_ids=list(range(N_CORES)),
                               **(_run_kw or {}))
    out = np.empty((B,), np.float32)
    for co in range(N_CORES):
        y = res.results[co]["yout"]                    # (2, 1024)
        base = co * B_CORE
        out[base:base + HB] = y[0]
        out[base + HB:base + B_CORE] = y[1]
    if _run_kw is not None:
        _CACHE["last_result"] = res
    return out + b0


# revision 11
# speedup vs baseline: 1.0505x; 1.0383x over previous
"""Trainium2 Bass kernel for nn_CompetitiveNetwork (competitive-binding solve).

Math (per batch column):
    K  = clip(exp(K_raw), 0, 1e3)   BT = clip(exp(BT_raw), 0, 1e3)
    fixed point:  u = 1/(1 + K^T AF);  AF = AT / (1 + (K diag(BT)) u)
    readout:      Y = sum_b (M^T AF)_b * u_b + b,  M = K*W*BT

Device algorithm (accelerated, validated numerically on host):
  state G (gain; AF = AT*G) and u. Per iteration:
    S  = W1blk @ af          (fp32r matmul, 128-part block-diag = 2 streams)
    u  = 1/(1+S)             (ACT reciprocal, exact)
    Tw = wg*W2blk @ u        (fp32r matmul; wg folded into weights)
    G  = ((1+wg) - (Tw+wg))*G)*G   one fused DVE op == SOR(wg) + Newton
         (iters 1..E_G use the exact path: ACT recip + scalar_tensor_tensor)
    af = AT*G                (DVE/Pool mul)
  9 over-relaxed iterations (wg=1.4) replace the reference's 21.5 plain
  iterations: SOR contraction ~0.33/iter vs 0.6, landing ~1.8e-3 from the
  reference iterate (fixed-point limit itself is only ~9e-5 away).
  All matmuls use fp32r (1 cyc/row at FD>=256; measured 9e-5 accurate on hw).

Sharding: pure data-parallel over batch (16384 -> 8 cores x 2048).
Layout: features on partitions, batch on free dim; two 64-partition streams
stacked into (128, FD) tiles; 4 column chunks of FD=256.
"""

import numpy as np

import concourse.bacc as bacc
import concourse.mybir as mybir
from concourse.tile import TileContext
from concourse.bass_utils import run_bass_kernel_spmd


# --- custom DVE op: NEWTON1P_ANT ---
# out = (c1 - (in0 + c0) * in1) * in1 : with (c0,c1)=(1,2) one Newton step
# of in1 toward 1/(1+in0); with (c0,c1)=(w,1+w) and in0=w*T it fuses the
# Newton step with SOR mixing: out = (1-w)*in1 + w*newton(in1; 1+T).

import concourse.dve_ops as dve_ops
from concourse.dve_ops import DveOp
from concourse.dve_spec import Spec, Src0, Src1, C0, C1, lower


def _ref_newton1p(in0, in1, c0, c1, c2):
    return ((c1 - (in0.astype(np.float32) + c0) * in1) * in1).astype(np.float32)


def _make_op(shas):
    return DveOp(
        "NEWTON1P_ANT",
        Spec(
            body=(C1 - (Src0 + C0) * Src1) * Src1,
            reference=_ref_newton1p,
        ),
        subdim=False,
        uops_sha=shas,
    )


def register():
    for op in dve_ops.OPS:
        if op.name == "NEWTON1P_ANT":
            return op
    probe = _make_op({})
    opcode = dve_ops._CUSTOM_DVE_ROW_BASE + len(dve_ops.OPS)
    shas = {}
    for ver in ("v3", "v4"):
        try:
            from concourse.dve_uop import DveOpSpec
            res = DveOpSpec(name=probe.name, opcode=opcode,
                            uops=lower(probe.spec, ver=ver),
                            rd1_en=True)
            shas[ver] = res.sha(ver)
        except Exception as e:
            print(f"lower {ver} failed: {e}")
    op = _make_op(shas)
    dve_ops.OPS.append(op)
    dve_ops.CUSTOM_DVE_SPECS[op.name] = op.spec
    dve_ops._SUB_OPCODE_FOR_NAME[op.name] = (
        dve_ops._CUSTOM_DVE_ROW_BASE + len(dve_ops.OPS) - 1)
    return op


def newton_sor(nc_vector, out, in0, in1, c0, c1):
    """out = (c1 - (in0 + c0) * in1) * in1 on the DVE."""
    op = register()
    return nc_vector._custom_dve(op, out=out, in0=in0, in1=in1,
                                 s0=float(c0), s1=float(c1), imm2=0.0)


B, NA, NB = 16384, 64, 64
N_CORES = 8
B_CORE = B // N_CORES          # 2048 batch columns per core
N_CHUNK = 4
FD = B_CORE // 2 // N_CHUNK    # 256

N_ITERS = 8                    # over-relaxed iterations
WG = 1.4                       # SOR factor on the G (gain) update
E_G = 3                        # iters 1..E_G use the exact ACT+stt G path

FP32 = mybir.dt.float32
FP32R = mybir.dt.float32r

_CACHE = {}


def _act_recip(nc, out_ap, in_ap, bias=1.0, scale=1.0):
    """out = 1/(in*scale + bias) on the Activation engine."""
    eng = nc.scalar
    ins = [eng.lower_ap(in_ap),
           mybir.ImmediateValue(dtype=FP32, value=float(bias)),
           mybir.ImmediateValue(dtype=FP32, value=float(scale)),
           mybir.ImmediateValue(dtype=FP32, value=0.0)]
    eng.add_instruction(mybir.InstActivation(
        name=nc.get_next_instruction_name(),
        func=mybir.ActivationFunctionType.Reciprocal,
        ins=ins, outs=[eng.lower_ap(out_ap)]))


def _build_module(repeat=1):
    register()
    nc = bacc.Bacc()
    # [W1blk | W2a | W2b | M2blk | ones2 | at0..at3]  (128, 514 + 1024)
    WCOL = 4 * 128 + 2
    ain = nc.dram_tensor("ain", (128, WCOL + N_CHUNK * FD), FP32,
                         kind="ExternalInput")
    yout = nc.dram_tensor("yout", (2, N_CHUNK * FD), FP32, kind="ExternalOutput")

    with TileContext(nc) as tc, \
         tc.tile_pool(name="const", bufs=1) as cpool, \
         tc.tile_pool(name="state", bufs=2) as spool, \
         tc.tile_pool(name="work", bufs=3) as wpool, \
         tc.tile_pool(name="psum", bufs=8, space="PSUM") as ppool:

        # two input DMAs on parallel DMA devices: weights+at0 via HWDGE
        # (SP queue), at1..3 via SWDGE (gpsimd queue)
        wa = cpool.tile([128, WCOL + FD], FP32, tag="wa")
        nc.sync.dma_start(out=wa[:], in_=ain[:, 0:WCOL + FD])
        a3 = cpool.tile([128, 3 * FD], FP32, tag="a3")
        nc.gpsimd.dma_start(out=a3[:], in_=ain[:, WCOL + FD:])
        wallr = cpool.tile([128, WCOL], FP32R, tag="wallr")
        nc.vector.tensor_copy(wallr[:], wa[:, 0:WCOL])
        w1f = wa[:, 0:128]             # fp32 W1 for the first iteration
        w1r = wallr[:, 0:128]
        w2ar = wallr[:, 128:256]
        w2br = wallr[:, 256:384]
        m2r = wallr[:, 384:512]
        onesr = wallr[:, 512:514]
        ats = [wa[:, WCOL:WCOL + FD]] +               [a3[:, i * FD:(i + 1) * FD] for i in range(3)]

        for _rep in range(repeat):
            af = list(ats)              # AF_0 = AT (G_0 = 1; fp32, used once)
            af_ap = [a for a in af]     # APs directly
            us = [None] * N_CHUNK
            gs = [None] * N_CHUNK
            banks = [None] * N_CHUNK

            def emit_halfstep(c, h):
                n = h // 2 + 1          # iteration number, 1-based
                if h % 2 == 0:
                    # S-side: ps = W1 af ; u = 1/(1+S) on ACT
                    # one PSUM bank per chunk-iteration: mm1 -> half 0,
                    # mm2 -> half 1 (PSUM tiles are bank-granular)
                    bank = ppool.tile([128, 2 * FD], FP32, tag="ps")
                    banks[c] = bank
                    ps = bank[:, 0:FD]
                    # iteration 1 runs in fp32 (af = AT straight from DMA);
                    # later iterations in fp32r (1 cyc/row)
                    rhs0 = af[c] if n == 1 else af[c][:]
                    nc.tensor.matmul(out=ps, lhsT=(w1f if n == 1 else w1r),
                                     rhs=rhs0, start=True, stop=True)
                    u_n = spool.tile([128, FD], FP32R, tag=f"u{c}")
                    _act_recip(nc, u_n[:], ps)
                    us[c] = u_n
                else:
                    # T-side: ps2 = wg*W2 u ; G update ; af = AT*G
                    ps2 = banks[c][:, FD:2 * FD]
                    nc.tensor.matmul(out=ps2, lhsT=(w2ar if n == 1 else w2br),
                                     rhs=us[c][:], start=True, stop=True)
                    g_n = spool.tile([128, FD], FP32, tag=f"g{c}")
                    if n == 1:
                        # G_1 = 1/(1+T)   (wg=1 first step)
                        _act_recip(nc, g_n[:], ps2)
                    elif n <= E_G:
                        # exact SOR: ACT emits R = wg/(1+T) directly
                        # (1/(ps2*s + b) with s=1/wg^2, b=1/wg on ps2=wg*T),
                        # then G = (1-wg)*G + R in one stt.
                        r_n = wpool.tile([128, FD], FP32, tag=f"r{c}")
                        _act_recip(nc, r_n[:], ps2,
                                   bias=1.0 / WG, scale=1.0 / (WG * WG))
                        nc.vector.scalar_tensor_tensor(
                            out=g_n[:], in0=gs[c][:], scalar=float(1.0 - WG),
                            in1=r_n[:], op0=mybir.AluOpType.mult,
                            op1=mybir.AluOpType.add)
                    else:
                        # fused Newton+SOR: G = ((1+wg)-(ps2+wg)*G)*G
                        newton_sor(nc.vector, g_n[:], ps2, gs[c][:],
                                   WG, 1.0 + WG)
                    gs[c] = g_n
                    af_n = spool.tile([128, FD], FP32R, tag=f"af{c}")
                    # alternate mul engine per iteration so no chunk always
                    # pays the slower Pool multiply
                    if (c + n) % 2 == 0:
                        nc.vector.tensor_mul(af_n[:], ats[c], gs[c][:])
                    else:
                        nc.gpsimd.tensor_mul(af_n[:], ats[c], gs[c][:])
                    af[c] = af_n

            # chunk c runs c half-steps behind chunk 0: four phases in
            # flight so every engine queue always holds ready work.
            H = 2 * N_ITERS
            for t in range(H + N_CHUNK):
                for c in range(N_CHUNK):
                    h = t - c
                    if 0 <= h < H:
                        emit_halfstep(c, h)

            # readout: S = W1 af; u = newton(u); V = M2 af; h = V*u;
            # Y = ones^T h. h packed pairwise -> one ones-matmul per pair.
            hpair0 = wpool.tile([128, 2 * FD], FP32R, tag="hp0")
            hpair1 = wpool.tile([128, 2 * FD], FP32R, tag="hp1")
            hpair = [hpair0, hpair1]
            for c in range(N_CHUNK):
                bank = ppool.tile([128, 2 * FD], FP32, tag="ps")
                ps = bank[:, 0:FD]
                pp = bank[:, FD:2 * FD]
                nc.tensor.matmul(out=ps, lhsT=w1r, rhs=af[c][:],
                                 start=True, stop=True)
                nc.tensor.matmul(out=pp, lhsT=m2r, rhs=af[c][:],
                                 start=True, stop=True)
                u1 = spool.tile([128, FD], FP32, tag=f"u{c}")
                newton_sor(nc.vector, u1[:], ps, us[c][:], 1.0, 2.0)
                hslot = hpair[c // 2][:, (c % 2) * FD:(c % 2 + 1) * FD]
                nc.vector.tensor_mul(hslot, pp, u1[:])
                if c % 2 == 1:
                    p = c // 2
                    ybank = ppool.tile([128, 2 * FD], FP32, tag="ps")
                    nc.tensor.matmul(out=ybank[0:2, :], lhsT=onesr,
                                     rhs=hpair[p][:], start=True, stop=True)
                    ys = wpool.tile([128, 2 * FD], FP32, tag=f"ys{p}")
                    nc.vector.tensor_copy(ys[0:2, :], ybank[0:2, :])
                    out_eng = nc.sync if p == 0 else nc.gpsimd
                    out_eng.dma_start(out=yout[:, p * 2 * FD:(p + 1) * 2 * FD],
                                      in_=ys[0:2, :])

    nc.finalize()
    return nc


def _get_module(repeat=1):
    key = f"nc{repeat}"
    if key not in _CACHE:
        _CACHE[key] = _build_module(repeat)
    return _CACHE[key]


def kernel(AT, K_raw, BT_raw, W_raw, b_raw, _run_kw=None, _repeat=1):
    AT = np.asarray(AT, dtype=np.float32)
    K = np.clip(np.exp(np.asarray(K_raw, np.float32)), 0.0, 1000.0).astype(np.float32)
    BT = np.clip(np.exp(np.asarray(BT_raw, np.float32)), 0.0, 1000.0).astype(np.float32)
    Wc = np.clip(np.asarray(W_raw, np.float32), -10.0, 10.0).reshape(NA, NB)
    b0 = np.clip(np.asarray(b_raw, np.float32), -10.0, 10.0)[0]

    w2 = np.ascontiguousarray((K * BT[None, :]).T)     # (nB,nA) lhsT: T = w2^T u
    M = K * Wc * BT[None, :]                           # (nA,nB) lhsT: V = M^T af

    def blk(a):
        z = np.zeros((128, 128), np.float32)
        z[0:64, 0:64] = a
        z[64:128, 64:128] = a
        return z

    ones2 = np.zeros((128, 2), np.float32)
    ones2[0:64, 0] = 1.0
    ones2[64:128, 1] = 1.0
    wts = np.concatenate([blk(K), blk(w2), blk(WG * w2), blk(M), ones2], axis=1)

    att = np.ascontiguousarray(AT.T)                   # (64, 16384)
    HB = B_CORE // 2                                   # 1024 cols per stream
    in_maps = []
    for c in range(N_CORES):
        chunk = att[:, c * B_CORE:(c + 1) * B_CORE]    # (64, 2048)
        stacked = np.concatenate([chunk[:, :HB], chunk[:, HB:]], axis=0)
        ain = np.ascontiguousarray(
            np.concatenate([wts, stacked], axis=1).astype(np.float32))
        in_maps.append({"ain": ain})

    nc = _get_module(_repeat)
    res = run_bass_kernel_spmd(nc, in_maps, core_ids=list(range(N_CORES)),
                               **(_run_kw or {}))
    out = np.empty((B,), np.float32)
    for co in range(N_CORES):
        y = res.results[co]["yout"]                    # (2, 1024)
        base = co * B_CORE
        out[base:base + HB] = y[0]
        out[base + HB:base + B_CORE] = y[1]
    if _run_kw is not None:
        _CACHE["last_result"] = res
    return out + b0


# revision 12
# speedup vs baseline: 1.0978x; 1.0450x over previous
"""Trainium2 Bass kernel for nn_CompetitiveNetwork (competitive-binding solve).

Math (per batch column):
    K  = clip(exp(K_raw), 0, 1e3)   BT = clip(exp(BT_raw), 0, 1e3)
    fixed point:  u = 1/(1 + K^T AF);  AF = AT / (1 + (K diag(BT)) u)
    readout:      Y = sum_b (M^T AF)_b * u_b + b,  M = K*W*BT

Device algorithm (accelerated, validated numerically on host):
  state G (gain; AF = AT*G) and u. Per iteration:
    S  = W1blk @ af          (fp32r matmul, 128-part block-diag = 2 streams)
    u  = 1/(1+S)             (ACT reciprocal, exact)
    Tw = wg*W2blk @ u        (fp32r matmul; wg folded into weights)
    G  = ((1+wg) - (Tw+wg))*G)*G   one fused DVE op == SOR(wg) + Newton
         (iters 1..E_G use the exact path: ACT recip + scalar_tensor_tensor)
    af = AT*G                (DVE/Pool mul)
  9 over-relaxed iterations (wg=1.4) replace the reference's 21.5 plain
  iterations: SOR contraction ~0.33/iter vs 0.6, landing ~1.8e-3 from the
  reference iterate (fixed-point limit itself is only ~9e-5 away).
  All matmuls use fp32r (1 cyc/row at FD>=256; measured 9e-5 accurate on hw).

Sharding: pure data-parallel over batch (16384 -> 8 cores x 2048).
Layout: features on partitions, batch on free dim; two 64-partition streams
stacked into (128, FD) tiles; 4 column chunks of FD=256.
"""

import numpy as np

import concourse.bacc as bacc
import concourse.mybir as mybir
from concourse.tile import TileContext
from concourse.bass_utils import run_bass_kernel_spmd


# --- custom DVE op: NEWTON1P_ANT ---
# out = (c1 - (in0 + c0) * in1) * in1 : with (c0,c1)=(1,2) one Newton step
# of in1 toward 1/(1+in0); with (c0,c1)=(w,1+w) and in0=w*T it fuses the
# Newton step with SOR mixing: out = (1-w)*in1 + w*newton(in1; 1+T).

import concourse.dve_ops as dve_ops
from concourse.dve_ops import DveOp
from concourse.dve_spec import Spec, Src0, Src1, C0, C1, lower


def _ref_newton1p(in0, in1, c0, c1, c2):
    return ((c1 - (in0.astype(np.float32) + c0) * in1) * in1).astype(np.float32)


def _make_op(shas):
    return DveOp(
        "NEWTON1P_ANT",
        Spec(
            body=(C1 - (Src0 + C0) * Src1) * Src1,
            reference=_ref_newton1p,
        ),
        subdim=False,
        uops_sha=shas,
    )


def register():
    for op in dve_ops.OPS:
        if op.name == "NEWTON1P_ANT":
            return op
    probe = _make_op({})
    opcode = dve_ops._CUSTOM_DVE_ROW_BASE + len(dve_ops.OPS)
    shas = {}
    for ver in ("v3", "v4"):
        try:
            from concourse.dve_uop import DveOpSpec
            res = DveOpSpec(name=probe.name, opcode=opcode,
                            uops=lower(probe.spec, ver=ver),
                            rd1_en=True)
            shas[ver] = res.sha(ver)
        except Exception as e:
            print(f"lower {ver} failed: {e}")
    op = _make_op(shas)
    dve_ops.OPS.append(op)
    dve_ops.CUSTOM_DVE_SPECS[op.name] = op.spec
    dve_ops._SUB_OPCODE_FOR_NAME[op.name] = (
        dve_ops._CUSTOM_DVE_ROW_BASE + len(dve_ops.OPS) - 1)
    return op


def newton_sor(nc_vector, out, in0, in1, c0, c1):
    """out = (c1 - (in0 + c0) * in1) * in1 on the DVE."""
    op = register()
    return nc_vector._custom_dve(op, out=out, in0=in0, in1=in1,
                                 s0=float(c0), s1=float(c1), imm2=0.0)


B, NA, NB = 16384, 64, 64
N_CORES = 8
B_CORE = B // N_CORES          # 2048 batch columns per core
N_CHUNK = 4
FD = B_CORE // 2 // N_CHUNK    # 256

N_ITERS = 8                    # over-relaxed iterations
WG = 1.4                       # SOR factor on the G (gain) update
E_G = 3                        # iters 1..E_G use the exact ACT+stt G path

FP32 = mybir.dt.float32
FP32R = mybir.dt.float32r

_CACHE = {}


def _act_recip(nc, out_ap, in_ap, bias=1.0, scale=1.0):
    """out = 1/(in*scale + bias) on the Activation engine."""
    eng = nc.scalar
    ins = [eng.lower_ap(in_ap),
           mybir.ImmediateValue(dtype=FP32, value=float(bias)),
           mybir.ImmediateValue(dtype=FP32, value=float(scale)),
           mybir.ImmediateValue(dtype=FP32, value=0.0)]
    eng.add_instruction(mybir.InstActivation(
        name=nc.get_next_instruction_name(),
        func=mybir.ActivationFunctionType.Reciprocal,
        ins=ins, outs=[eng.lower_ap(out_ap)]))


def _build_module(repeat=1):
    register()
    nc = bacc.Bacc()
    # [W1blk | W2a | W2b | M2blk | ones2 | at0..at3]  (128, 514 + 1024)
    WCOL = 4 * 128 + 2
    ain = nc.dram_tensor("ain", (128, WCOL + N_CHUNK * FD), FP32,
                         kind="ExternalInput")
    yout = nc.dram_tensor("yout", (2, N_CHUNK * FD), FP32, kind="ExternalOutput")

    with TileContext(nc) as tc, \
         tc.tile_pool(name="const", bufs=1) as cpool, \
         tc.tile_pool(name="state", bufs=2) as spool, \
         tc.tile_pool(name="work", bufs=3) as wpool, \
         tc.tile_pool(name="psum", bufs=8, space="PSUM") as ppool:

        # two input DMAs on parallel DMA devices: weights+at0 via HWDGE
        # (SP queue), at1..3 via SWDGE (gpsimd queue)
        wa = cpool.tile([128, WCOL + FD], FP32, tag="wa")
        nc.sync.dma_start(out=wa[:], in_=ain[:, 0:WCOL + FD])
        a3 = cpool.tile([128, 3 * FD], FP32, tag="a3")
        nc.gpsimd.dma_start(out=a3[:], in_=ain[:, WCOL + FD:])
        wallr = cpool.tile([128, WCOL], FP32R, tag="wallr")
        nc.vector.tensor_copy(wallr[:], wa[:, 0:WCOL])
        w1f = wa[:, 0:128]             # fp32 W1 for the first iteration
        w1r = wallr[:, 0:128]
        w2ar = wallr[:, 128:256]
        w2br = wallr[:, 256:384]
        m2r = wallr[:, 384:512]
        onesr = wallr[:, 512:514]
        ats = [wa[:, WCOL:WCOL + FD]] +               [a3[:, i * FD:(i + 1) * FD] for i in range(3)]

        for _rep in range(repeat):
            af = list(ats)              # AF_0 = AT (G_0 = 1; fp32, used once)
            af_ap = [a for a in af]     # APs directly
            us = [None] * N_CHUNK
            gs = [None] * N_CHUNK
            banks = [None] * N_CHUNK

            def emit_halfstep(c, h):
                n = h // 2 + 1          # iteration number, 1-based
                if h % 2 == 0:
                    # S-side: ps = W1 af ; u = 1/(1+S) on ACT
                    # one PSUM bank per chunk-iteration: mm1 -> half 0,
                    # mm2 -> half 1 (PSUM tiles are bank-granular)
                    bank = ppool.tile([128, 2 * FD], FP32, tag="ps")
                    banks[c] = bank
                    ps = bank[:, 0:FD]
                    # iteration 1 runs in fp32 (af = AT straight from DMA);
                    # later iterations in fp32r (1 cyc/row)
                    rhs0 = af[c] if n == 1 else af[c][:]
                    nc.tensor.matmul(out=ps, lhsT=(w1f if n == 1 else w1r),
                                     rhs=rhs0, start=True, stop=True)
                    u_n = spool.tile([128, FD], FP32R, tag=f"u{c}")
                    _act_recip(nc, u_n[:], ps)
                    us[c] = u_n
                else:
                    # T-side: ps2 = wg*W2 u ; G update ; af = AT*G
                    ps2 = banks[c][:, FD:2 * FD]
                    nc.tensor.matmul(out=ps2, lhsT=(w2ar if n == 1 else w2br),
                                     rhs=us[c][:], start=True, stop=True)
                    g_n = spool.tile([128, FD], FP32, tag=f"g{c}")
                    if n == 1:
                        # G_1 = 1/(1+T)   (wg=1 first step)
                        _act_recip(nc, g_n[:], ps2)
                    elif n <= E_G:
                        # exact SOR: ACT emits R = wg/(1+T) directly
                        # (1/(ps2*s + b) with s=1/wg^2, b=1/wg on ps2=wg*T),
                        # then G = (1-wg)*G + R in one stt.
                        r_n = wpool.tile([128, FD], FP32, tag=f"r{c}")
                        _act_recip(nc, r_n[:], ps2,
                                   bias=1.0 / WG, scale=1.0 / (WG * WG))
                        nc.vector.scalar_tensor_tensor(
                            out=g_n[:], in0=gs[c][:], scalar=float(1.0 - WG),
                            in1=r_n[:], op0=mybir.AluOpType.mult,
                            op1=mybir.AluOpType.add)
                    else:
                        # fused Newton+SOR: G = ((1+wg)-(ps2+wg)*G)*G
                        newton_sor(nc.vector, g_n[:], ps2, gs[c][:],
                                   WG, 1.0 + WG)
                    gs[c] = g_n
                    af_n = spool.tile([128, FD], FP32R, tag=f"af{c}")
                    # alternate mul engine per iteration so no chunk always
                    # pays the slower Pool multiply
                    if (c + n) % 2 == 0:
                        nc.vector.tensor_mul(af_n[:], ats[c], gs[c][:])
                    else:
                        nc.gpsimd.tensor_mul(af_n[:], ats[c], gs[c][:])
                    af[c] = af_n

            # chunk c runs c half-steps behind chunk 0: four phases in
            # flight so every engine queue always holds ready work.
            H = 2 * N_ITERS
            for t in range(H + N_CHUNK):
                for c in range(N_CHUNK):
                    h = t - c
                    if 0 <= h < H:
                        emit_halfstep(c, h)

            # readout: V = M2 af; h = V*u (u from the last S-side, half a
            # step stale -- validated numerically); Y = ones^T h.
            # h packed pairwise -> one ones-matmul per pair.
            hpair0 = wpool.tile([128, 2 * FD], FP32R, tag="hp0")
            hpair1 = wpool.tile([128, 2 * FD], FP32R, tag="hp1")
            hpair = [hpair0, hpair1]
            for c in range(N_CHUNK):
                bank = ppool.tile([128, 2 * FD], FP32, tag="ps")
                pp = bank[:, 0:FD]
                nc.tensor.matmul(out=pp, lhsT=m2r, rhs=af[c][:],
                                 start=True, stop=True)
                hslot = hpair[c // 2][:, (c % 2) * FD:(c % 2 + 1) * FD]
                nc.vector.tensor_mul(hslot, pp, us[c][:])
                if c % 2 == 1:
                    p = c // 2
                    ybank = ppool.tile([128, 2 * FD], FP32, tag="ps")
                    nc.tensor.matmul(out=ybank[0:2, :], lhsT=onesr,
                                     rhs=hpair[p][:], start=True, stop=True)
                    ys = wpool.tile([128, 2 * FD], FP32, tag=f"ys{p}")
                    nc.vector.tensor_copy(ys[0:2, :], ybank[0:2, :])
                    out_eng = nc.sync if p == 0 else nc.gpsimd
                    out_eng.dma_start(out=yout[:, p * 2 * FD:(p + 1) * 2 * FD],
                                      in_=ys[0:2, :])

    nc.finalize()
    return nc


def _get_module(repeat=1):
    key = f"nc{repeat}"
    if key not in _CACHE:
        _CACHE[key] = _build_module(repeat)
    return _CACHE[key]


def kernel(AT, K_raw, BT_raw, W_raw, b_raw, _run_kw=None, _repeat=1):
    AT = np.asarray(AT, dtype=np.float32)
    K = np.clip(np.exp(np.asarray(K_raw, np.float32)), 0.0, 1000.0).astype(np.float32)
    BT = np.clip(np.exp(np.asarray(BT_raw, np.float32)), 0.0, 1000.0).astype(np.float32)
    Wc = np.clip(np.asarray(W_raw, np.float32), -10.0, 10.0).reshape(NA, NB)
    b0 = np.clip(np.asarray(b_raw, np.float32), -10.0, 10.0)[0]

    w2 = np.ascontiguousarray((K * BT[None, :]).T)     # (nB,nA) lhsT: T = w2^T u
    M = K * Wc * BT[None, :]                           # (nA,nB) lhsT: V = M^T af

    def blk(a):
        z = np.zeros((128, 128), np.float32)
        z[0:64, 0:64] = a
        z[64:128, 64:128] = a
        return z

    ones2 = np.zeros((128, 2), np.float32)
    ones2[0:64, 0] = 1.0
    ones2[64:128, 1] = 1.0
    wts = np.concatenate([blk(K), blk(w2), blk(WG * w2), blk(M), ones2], axis=1)

    att = np.ascontiguousarray(AT.T)                   # (64, 16384)
    HB = B_CORE // 2                                   # 1024 cols per stream
    in_maps = []
    for c in range(N_CORES):
        chunk = att[:, c * B_CORE:(c + 1) * B_CORE]    # (64, 2048)
        stacked = np.concatenate([chunk[:, :HB], chunk[:, HB:]], axis=0)
        ain = np.ascontiguousarray(
            np.concatenate([wts, stacked], axis=1).astype(np.float32))
        in_maps.append({"ain": ain})

    nc = _get_module(_repeat)
    res = run_bass_kernel_spmd(nc, in_maps, core_ids=list(range(N_CORES)),
                               **(_run_kw or {}))
    out = np.empty((B,), np.float32)
    for co in range(N_CORES):
        y = res.results[co]["yout"]                    # (2, 1024)
        base = co * B_CORE
        out[base:base + HB] = y[0]
        out[base + HB:base + B_CORE] = y[1]
    if _run_kw is not None:
        _CACHE["last_result"] = res
    return out + b0


# revision 14
# speedup vs baseline: 1.2384x; 1.1281x over previous
"""Trainium2 Bass kernel for nn_CompetitiveNetwork (competitive-binding solve).

Math (per batch column):
    K  = clip(exp(K_raw), 0, 1e3)   BT = clip(exp(BT_raw), 0, 1e3)
    fixed point:  u = 1/(1 + K^T AF);  AF = AT / (1 + (K diag(BT)) u)
    readout:      Y = sum_b (M^T AF)_b * u_b + b,  M = K*W*BT

Device algorithm (accelerated, validated numerically on host):
  state G (gain; AF = AT*G) and u. Per iteration:
    S  = W1blk @ af          (fp32r matmul, 128-part block-diag = 2 streams)
    u  = 1/(1+S)             (ACT reciprocal, exact)
    Tw = wg*W2blk @ u        (fp32r matmul; wg folded into weights)
    G  = ((1+wg) - (Tw+wg))*G)*G   one fused DVE op == SOR(wg) + Newton
         (iters 1..E_G use the exact path: ACT recip + scalar_tensor_tensor)
    af = AT*G                (DVE/Pool mul)
  9 over-relaxed iterations (wg=1.4) replace the reference's 21.5 plain
  iterations: SOR contraction ~0.33/iter vs 0.6, landing ~1.8e-3 from the
  reference iterate (fixed-point limit itself is only ~9e-5 away).
  All matmuls use fp32r (1 cyc/row at FD>=256; measured 9e-5 accurate on hw).

Sharding: pure data-parallel over batch (16384 -> 8 cores x 2048).
Layout: features on partitions, batch on free dim; two 64-partition streams
stacked into (128, FD) tiles; 4 column chunks of FD=256.
"""

import numpy as np

import concourse.bacc as bacc
import concourse.mybir as mybir
from concourse.tile import TileContext
from concourse.bass_utils import run_bass_kernel_spmd


# --- custom DVE op: NEWTON1P_ANT ---
# out = (c1 - (in0 + c0) * in1) * in1 : with (c0,c1)=(1,2) one Newton step
# of in1 toward 1/(1+in0); with (c0,c1)=(w,1+w) and in0=w*T it fuses the
# Newton step with SOR mixing: out = (1-w)*in1 + w*newton(in1; 1+T).

import concourse.dve_ops as dve_ops
from concourse.dve_ops import DveOp
from concourse.dve_spec import Spec, Src0, Src1, C0, C1, lower


def _ref_newton1p(in0, in1, c0, c1, c2):
    return ((c1 - (in0.astype(np.float32) + c0) * in1) * in1).astype(np.float32)


def _make_op(shas):
    return DveOp(
        "NEWTON1P_ANT",
        Spec(
            body=(C1 - (Src0 + C0) * Src1) * Src1,
            reference=_ref_newton1p,
        ),
        subdim=False,
        uops_sha=shas,
    )


def register():
    for op in dve_ops.OPS:
        if op.name == "NEWTON1P_ANT":
            return op
    probe = _make_op({})
    opcode = dve_ops._CUSTOM_DVE_ROW_BASE + len(dve_ops.OPS)
    shas = {}
    for ver in ("v3", "v4"):
        try:
            from concourse.dve_uop import DveOpSpec
            res = DveOpSpec(name=probe.name, opcode=opcode,
                            uops=lower(probe.spec, ver=ver),
                            rd1_en=True)
            shas[ver] = res.sha(ver)
        except Exception as e:
            print(f"lower {ver} failed: {e}")
    op = _make_op(shas)
    dve_ops.OPS.append(op)
    dve_ops.CUSTOM_DVE_SPECS[op.name] = op.spec
    dve_ops._SUB_OPCODE_FOR_NAME[op.name] = (
        dve_ops._CUSTOM_DVE_ROW_BASE + len(dve_ops.OPS) - 1)
    return op


def newton_sor(nc_vector, out, in0, in1, c0, c1):
    """out = (c1 - (in0 + c0) * in1) * in1 on the DVE."""
    op = register()
    return nc_vector._custom_dve(op, out=out, in0=in0, in1=in1,
                                 s0=float(c0), s1=float(c1), imm2=0.0)


B, NA, NB = 16384, 64, 64
N_CORES = 8
B_CORE = B // N_CORES          # 2048 batch columns per core
N_CHUNK = 4
FD = B_CORE // 2 // N_CHUNK    # 256

N_ITERS = 8                    # over-relaxed iterations
WG = 1.4                       # SOR factor on the G (gain) update
E_G = 3                        # iters 1..E_G use the exact ACT+stt G path

FP32 = mybir.dt.float32
FP32R = mybir.dt.float32r
FP16 = mybir.dt.float16

_CACHE = {}


def _act_recip(nc, out_ap, in_ap, bias=1.0, scale=1.0):
    """out = 1/(in*scale + bias) on the Activation engine."""
    eng = nc.scalar
    ins = [eng.lower_ap(in_ap),
           mybir.ImmediateValue(dtype=FP32, value=float(bias)),
           mybir.ImmediateValue(dtype=FP32, value=float(scale)),
           mybir.ImmediateValue(dtype=FP32, value=0.0)]
    eng.add_instruction(mybir.InstActivation(
        name=nc.get_next_instruction_name(),
        func=mybir.ActivationFunctionType.Reciprocal,
        ins=ins, outs=[eng.lower_ap(out_ap)]))


def _build_module(repeat=1):
    register()
    nc = bacc.Bacc()
    # [W1blk | W2b | M2blk | ones2 | at0..at3]  fp16 over the wire
    WCOL = 3 * 128 + 2
    ain = nc.dram_tensor("ain", (128, WCOL + N_CHUNK * FD), FP16,
                         kind="ExternalInput")
    yout = nc.dram_tensor("yout", (2, N_CHUNK * FD), FP32, kind="ExternalOutput")

    with TileContext(nc) as tc, \
         tc.tile_pool(name="const", bufs=1) as cpool, \
         tc.tile_pool(name="state", bufs=2) as spool, \
         tc.tile_pool(name="work", bufs=3) as wpool, \
         tc.tile_pool(name="psum", bufs=8, space="PSUM") as ppool:

        # two input DMAs on parallel DMA devices: weights+at0 via HWDGE
        # (SP queue), at1..3 via SWDGE (gpsimd queue); fp16 over the wire
        wa = cpool.tile([128, WCOL + FD], FP16, tag="wa")
        nc.sync.dma_start(out=wa[:], in_=ain[:, 0:WCOL + FD])
        a3 = cpool.tile([128, 3 * FD], FP16, tag="a3")
        nc.gpsimd.dma_start(out=a3[:], in_=ain[:, WCOL + FD:])
        wallr = cpool.tile([128, WCOL], FP32R, tag="wallr")
        nc.vector.tensor_copy(wallr[:], wa[:, 0:WCOL])
        w1h = wa[:, 0:128]             # fp16 W1 for the first iteration
        w1r = wallr[:, 0:128]
        w2br = wallr[:, 128:256]
        m2r = wallr[:, 256:384]
        onesr = wallr[:, 384:386]
        ats = [wa[:, WCOL:WCOL + FD]] + \
              [a3[:, i * FD:(i + 1) * FD] for i in range(3)]

        for _rep in range(repeat):
            af = list(ats)              # AF_0 = AT (G_0 = 1; fp32, used once)
            af_ap = [a for a in af]     # APs directly
            us = [None] * N_CHUNK
            gs = [None] * N_CHUNK
            banks = [None] * N_CHUNK

            def emit_halfstep(c, h):
                n = h // 2 + 1          # iteration number, 1-based
                if h % 2 == 0:
                    # S-side: ps = W1 af ; u = 1/(1+S) on ACT
                    # one PSUM bank per chunk-iteration: mm1 -> half 0,
                    # mm2 -> half 1 (PSUM tiles are bank-granular)
                    bank = ppool.tile([128, 2 * FD], FP32, tag="ps")
                    banks[c] = bank
                    ps = bank[:, 0:FD]
                    # iteration 1 runs in fp32 (af = AT straight from DMA);
                    # later iterations in fp32r (1 cyc/row)
                    rhs0 = af[c] if n == 1 else af[c][:]
                    nc.tensor.matmul(out=ps, lhsT=(w1h if n == 1 else w1r),
                                     rhs=rhs0, start=True, stop=True)
                    u_n = spool.tile([128, FD], FP32R, tag=f"u{c}")
                    _act_recip(nc, u_n[:], ps)
                    us[c] = u_n
                else:
                    # T-side: ps2 = wg*W2 u ; G update ; af = AT*G
                    ps2 = banks[c][:, FD:2 * FD]
                    nc.tensor.matmul(out=ps2, lhsT=w2br,
                                     rhs=us[c][:], start=True, stop=True)
                    g_n = spool.tile([128, FD], FP32, tag=f"g{c}")
                    if n == 1:
                        # G_1 = 1/(1+T): ps2 = wg*T, so scale by 1/wg
                        _act_recip(nc, g_n[:], ps2, bias=1.0, scale=1.0 / WG)
                    elif n <= E_G:
                        # exact SOR: ACT emits R = wg/(1+T) directly
                        # (1/(ps2*s + b) with s=1/wg^2, b=1/wg on ps2=wg*T),
                        # then G = (1-wg)*G + R in one stt.
                        r_n = wpool.tile([128, FD], FP32, tag=f"r{c}")
                        _act_recip(nc, r_n[:], ps2,
                                   bias=1.0 / WG, scale=1.0 / (WG * WG))
                        nc.vector.scalar_tensor_tensor(
                            out=g_n[:], in0=gs[c][:], scalar=float(1.0 - WG),
                            in1=r_n[:], op0=mybir.AluOpType.mult,
                            op1=mybir.AluOpType.add)
                    else:
                        # fused Newton+SOR: G = ((1+wg)-(ps2+wg)*G)*G
                        newton_sor(nc.vector, g_n[:], ps2, gs[c][:],
                                   WG, 1.0 + WG)
                    gs[c] = g_n
                    af_n = spool.tile([128, FD], FP32R, tag=f"af{c}")
                    # alternate mul engine per iteration so no chunk always
                    # pays the slower Pool multiply
                    if (c + n) % 2 == 0:
                        nc.vector.tensor_mul(af_n[:], ats[c], gs[c][:])
                    else:
                        nc.gpsimd.tensor_mul(af_n[:], ats[c], gs[c][:])
                    af[c] = af_n

            # chunk c runs c half-steps behind chunk 0: four phases in
            # flight so every engine queue always holds ready work.
            H = 2 * N_ITERS
            for t in range(H + N_CHUNK):
                for c in range(N_CHUNK):
                    h = t - c
                    if 0 <= h < H:
                        emit_halfstep(c, h)

            # readout: V = M2 af; h = V*u (u from the last S-side, half a
            # step stale -- validated numerically); Y = ones^T h.
            # h packed pairwise -> one ones-matmul per pair.
            hpair0 = wpool.tile([128, 2 * FD], FP32R, tag="hp0")
            hpair1 = wpool.tile([128, 2 * FD], FP32R, tag="hp1")
            hpair = [hpair0, hpair1]
            for c in range(N_CHUNK):
                bank = ppool.tile([128, 2 * FD], FP32, tag="ps")
                pp = bank[:, 0:FD]
                nc.tensor.matmul(out=pp, lhsT=m2r, rhs=af[c][:],
                                 start=True, stop=True)
                hslot = hpair[c // 2][:, (c % 2) * FD:(c % 2 + 1) * FD]
                nc.vector.tensor_mul(hslot, pp, us[c][:])
                if c % 2 == 1:
                    p = c // 2
                    ybank = ppool.tile([128, 2 * FD], FP32, tag="ps")
                    nc.tensor.matmul(out=ybank[0:2, :], lhsT=onesr,
                                     rhs=hpair[p][:], start=True, stop=True)
                    ys = wpool.tile([128, 2 * FD], FP32, tag=f"ys{p}")
                    nc.scalar.copy(ys[0:2, :], ybank[0:2, :])
                    nc.sync.dma_start(out=yout[:, p * 2 * FD:(p + 1) * 2 * FD],
                                      in_=ys[0:2, :])

    nc.finalize()
    return nc


def _get_module(repeat=1):
    key = f"nc{repeat}"
    if key not in _CACHE:
        _CACHE[key] = _build_module(repeat)
    return _CACHE[key]


def kernel(AT, K_raw, BT_raw, W_raw, b_raw, _run_kw=None, _repeat=1):
    AT = np.asarray(AT, dtype=np.float32)
    K = np.clip(np.exp(np.asarray(K_raw, np.float32)), 0.0, 1000.0).astype(np.float32)
    BT = np.clip(np.exp(np.asarray(BT_raw, np.float32)), 0.0, 1000.0).astype(np.float32)
    Wc = np.clip(np.asarray(W_raw, np.float32), -10.0, 10.0).reshape(NA, NB)
    b0 = np.clip(np.asarray(b_raw, np.float32), -10.0, 10.0)[0]

    w2 = np.ascontiguousarray((K * BT[None, :]).T)     # (nB,nA) lhsT: T = w2^T u
    M = K * Wc * BT[None, :]                           # (nA,nB) lhsT: V = M^T af

    def blk(a):
        z = np.zeros((128, 128), np.float32)
        z[0:64, 0:64] = a
        z[64:128, 64:128] = a
        return z

    ones2 = np.zeros((128, 2), np.float32)
    ones2[0:64, 0] = 1.0
    ones2[64:128, 1] = 1.0
    wts = np.concatenate([blk(K), blk(WG * w2), blk(M), ones2], axis=1)

    att = np.ascontiguousarray(AT.T)                   # (64, 16384)
    HB = B_CORE // 2                                   # 1024 cols per stream
    in_maps = []
    for c in range(N_CORES):
        chunk = att[:, c * B_CORE:(c + 1) * B_CORE]    # (64, 2048)
        stacked = np.concatenate([chunk[:, :HB], chunk[:, HB:]], axis=0)
        ain = np.ascontiguousarray(
            np.concatenate([wts, stacked], axis=1).astype(np.float16))
        in_maps.append({"ain": ain})

    nc = _get_module(_repeat)
    res = run_bass_kernel_spmd(nc, in_maps, core_ids=list(range(N_CORES)),
                               **(_run_kw or {}))
    out = np.empty((B,), np.float32)
    for co in range(N_CORES):
        y = res.results[co]["yout"]                    # (2, 1024)
        base = co * B_CORE
        out[base:base + HB] = y[0]
        out[base + HB:base + B_CORE] = y[1]
    if _run_kw is not None:
        _CACHE["last_result"] = res
    return out + b0


# revision 16
# speedup vs baseline: 1.2688x; 1.0245x over previous
"""Trainium2 Bass kernel for nn_CompetitiveNetwork (competitive-binding solve).

Math (per batch column):
    K  = clip(exp(K_raw), 0, 1e3)   BT = clip(exp(BT_raw), 0, 1e3)
    fixed point:  u = 1/(1 + K^T AF);  AF = AT / (1 + (K diag(BT)) u)
    readout:      Y = sum_b (M^T AF)_b * u_b + b,  M = K*W*BT

Device algorithm (accelerated, validated numerically on host):
  state G (gain; AF = AT*G) and u. Per iteration:
    S  = W1blk @ af          (fp32r matmul, 128-part block-diag = 2 streams)
    u  = 1/(1+S)             (ACT reciprocal, exact)
    Tw = wg*W2blk @ u        (fp32r matmul; wg folded into weights)
    G  = ((1+wg) - (Tw+wg))*G)*G   one fused DVE op == SOR(wg) + Newton
         (iters 1..E_G use the exact path: ACT recip + scalar_tensor_tensor)
    af = AT*G                (DVE/Pool mul)
  9 over-relaxed iterations (wg=1.4) replace the reference's 21.5 plain
  iterations: SOR contraction ~0.33/iter vs 0.6, landing ~1.8e-3 from the
  reference iterate (fixed-point limit itself is only ~9e-5 away).
  All matmuls use fp32r (1 cyc/row at FD>=256; measured 9e-5 accurate on hw).

Sharding: pure data-parallel over batch (16384 -> 8 cores x 2048).
Layout: features on partitions, batch on free dim; two 64-partition streams
stacked into (128, FD) tiles; 4 column chunks of FD=256.
"""

import numpy as np

import concourse.bacc as bacc
import concourse.mybir as mybir
from concourse.tile import TileContext
from concourse.bass_utils import run_bass_kernel_spmd


# --- custom DVE op: NEWTON1P_ANT ---
# out = (c1 - (in0 + c0) * in1) * in1 : with (c0,c1)=(1,2) one Newton step
# of in1 toward 1/(1+in0); with (c0,c1)=(w,1+w) and in0=w*T it fuses the
# Newton step with SOR mixing: out = (1-w)*in1 + w*newton(in1; 1+T).

import concourse.dve_ops as dve_ops
from concourse.dve_ops import DveOp
from concourse.dve_spec import Spec, Src0, Src1, C0, C1, lower


def _ref_newton1p(in0, in1, c0, c1, c2):
    return ((c1 - (in0.astype(np.float32) + c0) * in1) * in1).astype(np.float32)


def _make_op(shas):
    return DveOp(
        "NEWTON1P_ANT",
        Spec(
            body=(C1 - (Src0 + C0) * Src1) * Src1,
            reference=_ref_newton1p,
        ),
        subdim=False,
        uops_sha=shas,
    )


def register():
    for op in dve_ops.OPS:
        if op.name == "NEWTON1P_ANT":
            return op
    probe = _make_op({})
    opcode = dve_ops._CUSTOM_DVE_ROW_BASE + len(dve_ops.OPS)
    shas = {}
    for ver in ("v3", "v4"):
        try:
            from concourse.dve_uop import DveOpSpec
            res = DveOpSpec(name=probe.name, opcode=opcode,
                            uops=lower(probe.spec, ver=ver),
                            rd1_en=True)
            shas[ver] = res.sha(ver)
        except Exception as e:
            print(f"lower {ver} failed: {e}")
    op = _make_op(shas)
    dve_ops.OPS.append(op)
    dve_ops.CUSTOM_DVE_SPECS[op.name] = op.spec
    dve_ops._SUB_OPCODE_FOR_NAME[op.name] = (
        dve_ops._CUSTOM_DVE_ROW_BASE + len(dve_ops.OPS) - 1)
    return op


def newton_sor(nc_vector, out, in0, in1, c0, c1):
    """out = (c1 - (in0 + c0) * in1) * in1 on the DVE."""
    op = register()
    return nc_vector._custom_dve(op, out=out, in0=in0, in1=in1,
                                 s0=float(c0), s1=float(c1), imm2=0.0)


B, NA, NB = 16384, 64, 64
N_CORES = 8
B_CORE = B // N_CORES          # 2048 batch columns per core
N_CHUNK = 4
FD = B_CORE // 2 // N_CHUNK    # 256

N_ITERS = 8                    # over-relaxed iterations
WG = 1.4                       # SOR factor on the G (gain) update
E_G = 3                        # iters 1..E_G use the exact ACT+stt G path

FP32 = mybir.dt.float32
FP32R = mybir.dt.float32r
FP16 = mybir.dt.float16

_CACHE = {}


def _act_recip(nc, out_ap, in_ap, bias=1.0, scale=1.0):
    """out = 1/(in*scale + bias) on the Activation engine."""
    eng = nc.scalar
    ins = [eng.lower_ap(in_ap),
           mybir.ImmediateValue(dtype=FP32, value=float(bias)),
           mybir.ImmediateValue(dtype=FP32, value=float(scale)),
           mybir.ImmediateValue(dtype=FP32, value=0.0)]
    eng.add_instruction(mybir.InstActivation(
        name=nc.get_next_instruction_name(),
        func=mybir.ActivationFunctionType.Reciprocal,
        ins=ins, outs=[eng.lower_ap(out_ap)]))


def _build_module(repeat=1):
    register()
    nc = bacc.Bacc()
    # [W1blk | W2b | M2blk | ones2 | at0..at3]  fp16 over the wire
    WCOL = 3 * 128 + 2
    ain = nc.dram_tensor("ain", (128, WCOL + N_CHUNK * FD), FP16,
                         kind="ExternalInput")
    yout = nc.dram_tensor("yout", (2, N_CHUNK * FD), FP32, kind="ExternalOutput")

    with TileContext(nc) as tc, \
         tc.tile_pool(name="const", bufs=1) as cpool, \
         tc.tile_pool(name="state", bufs=2) as spool, \
         tc.tile_pool(name="work", bufs=3) as wpool, \
         tc.tile_pool(name="psum", bufs=8, space="PSUM") as ppool:

        # two input DMAs on parallel DMA devices: weights+at0 via HWDGE
        # (SP queue), at1..3 via SWDGE (gpsimd queue); fp16 over the wire
        wa = cpool.tile([128, WCOL + FD], FP16, tag="wa")
        nc.sync.dma_start(out=wa[:], in_=ain[:, 0:WCOL + FD])
        a3 = cpool.tile([128, 3 * FD], FP16, tag="a3")
        nc.gpsimd.dma_start(out=a3[:], in_=ain[:, WCOL + FD:])
        wallr = cpool.tile([128, WCOL], FP32R, tag="wallr")
        nc.vector.tensor_copy(wallr[:], wa[:, 0:WCOL])
        w1h = wa[:, 0:128]             # fp16 W1 for the first iteration
        w1r = wallr[:, 0:128]
        w2br = wallr[:, 128:256]
        m2r = wallr[:, 256:384]
        onesr = wallr[:, 384:386]
        ats = [wa[:, WCOL:WCOL + FD]] + \
              [a3[:, i * FD:(i + 1) * FD] for i in range(3)]

        for _rep in range(repeat):
            af = list(ats)              # AF_0 = AT (G_0 = 1; fp32, used once)
            af_ap = [a for a in af]     # APs directly
            us = [None] * N_CHUNK
            gs = [None] * N_CHUNK
            banks = [None] * N_CHUNK

            def emit_halfstep(c, h):
                n = h // 2 + 1          # iteration number, 1-based
                if h % 2 == 0:
                    # S-side: ps = W1 af ; u = 1/(1+S) on ACT
                    # one PSUM bank per chunk-iteration: mm1 -> half 0,
                    # mm2 -> half 1 (PSUM tiles are bank-granular)
                    bank = ppool.tile([128, 2 * FD], FP32, tag="ps")
                    banks[c] = bank
                    ps = bank[:, 0:FD]
                    # iteration 1 runs in fp32 (af = AT straight from DMA);
                    # later iterations in fp32r (1 cyc/row)
                    rhs0 = af[c] if n == 1 else af[c][:]
                    nc.tensor.matmul(out=ps, lhsT=(w1h if n == 1 else w1r),
                                     rhs=rhs0, start=True, stop=True)
                    u_n = spool.tile([128, FD], FP32R, tag=f"u{c}")
                    _act_recip(nc, u_n[:], ps)
                    us[c] = u_n
                else:
                    # T-side: ps2 = wg*W2 u ; G update ; af = AT*G
                    ps2 = banks[c][:, FD:2 * FD]
                    nc.tensor.matmul(out=ps2, lhsT=w2br,
                                     rhs=us[c][:], start=True, stop=True)
                    g_n = spool.tile([128, FD], FP32, tag=f"g{c}")
                    if n == 1:
                        # G_1 = 1/(1+T): ps2 = wg*T, so scale by 1/wg
                        _act_recip(nc, g_n[:], ps2, bias=1.0, scale=1.0 / WG)
                    elif n <= E_G:
                        # exact SOR: ACT emits R = wg/(1+T) directly
                        # (1/(ps2*s + b) with s=1/wg^2, b=1/wg on ps2=wg*T),
                        # then G = (1-wg)*G + R in one stt.
                        r_n = wpool.tile([128, FD], FP32, tag=f"r{c}")
                        _act_recip(nc, r_n[:], ps2,
                                   bias=1.0 / WG, scale=1.0 / (WG * WG))
                        nc.vector.scalar_tensor_tensor(
                            out=g_n[:], in0=gs[c][:], scalar=float(1.0 - WG),
                            in1=r_n[:], op0=mybir.AluOpType.mult,
                            op1=mybir.AluOpType.add)
                    else:
                        # fused Newton+SOR: G = ((1+wg)-(ps2+wg)*G)*G
                        newton_sor(nc.vector, g_n[:], ps2, gs[c][:],
                                   WG, 1.0 + WG)
                    gs[c] = g_n
                    af_n = spool.tile([128, FD], FP32R, tag=f"af{c}")
                    # mul on DVE right behind the G custom: same in-order
                    # queue, no cross-engine semaphore hop
                    nc.vector.tensor_mul(af_n[:], ats[c], gs[c][:])
                    af[c] = af_n

            # chunk c runs c half-steps behind chunk 0: four phases in
            # flight so every engine queue always holds ready work.
            H = 2 * N_ITERS
            for t in range(H + N_CHUNK):
                for c in range(N_CHUNK):
                    h = t - c
                    if 0 <= h < H:
                        emit_halfstep(c, h)

            # readout: V = M2 af; h = V*u (u from the last S-side, half a
            # step stale -- validated numerically); Y = ones^T h.
            # h packed pairwise -> one ones-matmul per pair.
            hpair0 = wpool.tile([128, 2 * FD], FP32R, tag="hp0")
            hpair1 = wpool.tile([128, 2 * FD], FP32R, tag="hp1")
            hpair = [hpair0, hpair1]
            ysall = wpool.tile([128, 4 * FD], FP32, tag="ysall")
            for c in range(N_CHUNK):
                bank = ppool.tile([128, 2 * FD], FP32, tag="ps")
                pp = bank[:, 0:FD]
                nc.tensor.matmul(out=pp, lhsT=m2r, rhs=af[c][:],
                                 start=True, stop=True)
                hslot = hpair[c // 2][:, (c % 2) * FD:(c % 2 + 1) * FD]
                nc.vector.tensor_mul(hslot, pp, us[c][:])
                if c % 2 == 1:
                    p = c // 2
                    ybank = ppool.tile([128, 2 * FD], FP32, tag="ps")
                    nc.tensor.matmul(out=ybank[0:2, :], lhsT=onesr,
                                     rhs=hpair[p][:], start=True, stop=True)
                    nc.scalar.copy(ysall[0:2, p * 2 * FD:(p + 1) * 2 * FD],
                                   ybank[0:2, :])
            nc.sync.dma_start(out=yout[:, :], in_=ysall[0:2, :])

    nc.finalize()
    return nc


def _get_module(repeat=1):
    key = f"nc{repeat}"
    if key not in _CACHE:
        _CACHE[key] = _build_module(repeat)
    return _CACHE[key]


def kernel(AT, K_raw, BT_raw, W_raw, b_raw, _run_kw=None, _repeat=1):
    AT = np.asarray(AT, dtype=np.float32)
    K = np.clip(np.exp(np.asarray(K_raw, np.float32)), 0.0, 1000.0).astype(np.float32)
    BT = np.clip(np.exp(np.asarray(BT_raw, np.float32)), 0.0, 1000.0).astype(np.float32)
    Wc = np.clip(np.asarray(W_raw, np.float32), -10.0, 10.0).reshape(NA, NB)
    b0 = np.clip(np.asarray(b_raw, np.float32), -10.0, 10.0)[0]

    w2 = np.ascontiguousarray((K * BT[None, :]).T)     # (nB,nA) lhsT: T = w2^T u
    M = K * Wc * BT[None, :]                           # (nA,nB) lhsT: V = M^T af

    def blk(a):
        z = np.zeros((128, 128), np.float32)
        z[0:64, 0:64] = a
        z[64:128, 64:128] = a
        return z

    ones2 = np.zeros((128, 2), np.float32)
    ones2[0:64, 0] = 1.0
    ones2[64:128, 1] = 1.0
    wts = np.concatenate([blk(K), blk(WG * w2), blk(M), ones2], axis=1)

    att = np.ascontiguousarray(AT.T)                   # (64, 16384)
    HB = B_CORE // 2                                   # 1024 cols per stream
    in_maps = []
    for c in range(N_CORES):
        chunk = att[:, c * B_CORE:(c + 1) * B_CORE]    # (64, 2048)
        stacked = np.concatenate([chunk[:, :HB], chunk[:, HB:]], axis=0)
        ain = np.ascontiguousarray(
            np.concatenate([wts, stacked], axis=1).astype(np.float16))
        in_maps.append({"ain": ain})

    nc = _get_module(_repeat)
    res = run_bass_kernel_spmd(nc, in_maps, core_ids=list(range(N_CORES)),
                               **(_run_kw or {}))
    out = np.empty((B,), np.float32)
    for co in range(N_CORES):
        y = res.results[co]["yout"]                    # (2, 1024)
        base = co * B_CORE
        out[base:base + HB] = y[0]
        out[base + HB:base + B_CORE] = y[1]
    if _run_kw is not None:
        _CACHE["last_result"] = res
    return out + b0


# revision 17
# speedup vs baseline: 1.3779x; 1.0860x over previous
"""Trainium2 Bass kernel for nn_CompetitiveNetwork (competitive-binding solve).

Math (per batch column):
    K  = clip(exp(K_raw), 0, 1e3)   BT = clip(exp(BT_raw), 0, 1e3)
    fixed point:  u = 1/(1 + K^T AF);  AF = AT / (1 + (K diag(BT)) u)
    readout:      Y = sum_b (M^T AF)_b * u_b + b,  M = K*W*BT

Device algorithm (accelerated, validated numerically on host):
  state G (gain; AF = AT*G) and u. Per iteration:
    S  = W1blk @ af          (fp32r matmul, 128-part block-diag = 2 streams)
    u  = 1/(1+S)             (ACT reciprocal, exact)
    Tw = wg*W2blk @ u        (fp32r matmul; wg folded into weights)
    G  = ((1+wg) - (Tw+wg))*G)*G   one fused DVE op == SOR(wg) + Newton
         (iters 1..E_G use the exact path: ACT recip + scalar_tensor_tensor)
    af = AT*G                (DVE/Pool mul)
  9 over-relaxed iterations (wg=1.4) replace the reference's 21.5 plain
  iterations: SOR contraction ~0.33/iter vs 0.6, landing ~1.8e-3 from the
  reference iterate (fixed-point limit itself is only ~9e-5 away).
  All matmuls use fp32r (1 cyc/row at FD>=256; measured 9e-5 accurate on hw).

Sharding: pure data-parallel over batch (16384 -> 8 cores x 2048).
Layout: features on partitions, batch on free dim; two 64-partition streams
stacked into (128, FD) tiles; 4 column chunks of FD=256.
"""

import numpy as np

import concourse.bacc as bacc
import concourse.mybir as mybir
from concourse.tile import TileContext
from concourse.bass_utils import run_bass_kernel_spmd


# --- custom DVE op: NEWTON1P_ANT ---
# out = (c1 - (in0 + c0) * in1) * in1 : with (c0,c1)=(1,2) one Newton step
# of in1 toward 1/(1+in0); with (c0,c1)=(w,1+w) and in0=w*T it fuses the
# Newton step with SOR mixing: out = (1-w)*in1 + w*newton(in1; 1+T).

import concourse.dve_ops as dve_ops
from concourse.dve_ops import DveOp
from concourse.dve_spec import Spec, Src0, Src1, C0, C1, lower


def _ref_newton1p(in0, in1, c0, c1, c2):
    return ((c1 - (in0.astype(np.float32) + c0) * in1) * in1).astype(np.float32)


def _make_op(shas):
    return DveOp(
        "NEWTON1P_ANT",
        Spec(
            body=(C1 - (Src0 + C0) * Src1) * Src1,
            reference=_ref_newton1p,
        ),
        subdim=False,
        uops_sha=shas,
    )


def register():
    for op in dve_ops.OPS:
        if op.name == "NEWTON1P_ANT":
            return op
    probe = _make_op({})
    opcode = dve_ops._CUSTOM_DVE_ROW_BASE + len(dve_ops.OPS)
    shas = {}
    for ver in ("v3", "v4"):
        try:
            from concourse.dve_uop import DveOpSpec
            res = DveOpSpec(name=probe.name, opcode=opcode,
                            uops=lower(probe.spec, ver=ver),
                            rd1_en=True)
            shas[ver] = res.sha(ver)
        except Exception as e:
            print(f"lower {ver} failed: {e}")
    op = _make_op(shas)
    dve_ops.OPS.append(op)
    dve_ops.CUSTOM_DVE_SPECS[op.name] = op.spec
    dve_ops._SUB_OPCODE_FOR_NAME[op.name] = (
        dve_ops._CUSTOM_DVE_ROW_BASE + len(dve_ops.OPS) - 1)
    return op


def newton_sor(nc_vector, out, in0, in1, c0, c1):
    """out = (c1 - (in0 + c0) * in1) * in1 on the DVE."""
    op = register()
    return nc_vector._custom_dve(op, out=out, in0=in0, in1=in1,
                                 s0=float(c0), s1=float(c1), imm2=0.0)


B, NA, NB = 16384, 64, 64
N_CORES = 8
B_CORE = B // N_CORES          # 2048 batch columns per core
N_CHUNK = 4
FD = B_CORE // 2 // N_CHUNK    # 256

N_ITERS = 7                    # over-relaxed iterations
WG = 1.35                      # SOR factor on the G (gain) update
E_G = 3                        # iters 1..E_G use the exact ACT+stt G path

FP32 = mybir.dt.float32
FP32R = mybir.dt.float32r
FP16 = mybir.dt.float16

_CACHE = {}


def _act_recip(nc, out_ap, in_ap, bias=1.0, scale=1.0):
    """out = 1/(in*scale + bias) on the Activation engine."""
    eng = nc.scalar
    ins = [eng.lower_ap(in_ap),
           mybir.ImmediateValue(dtype=FP32, value=float(bias)),
           mybir.ImmediateValue(dtype=FP32, value=float(scale)),
           mybir.ImmediateValue(dtype=FP32, value=0.0)]
    eng.add_instruction(mybir.InstActivation(
        name=nc.get_next_instruction_name(),
        func=mybir.ActivationFunctionType.Reciprocal,
        ins=ins, outs=[eng.lower_ap(out_ap)]))


def _build_module(repeat=1):
    register()
    nc = bacc.Bacc()
    # [W1blk | W2b | M2blk | ones2 | at0..at3]  fp16 over the wire
    WCOL = 3 * 128 + 2
    ain = nc.dram_tensor("ain", (128, WCOL + N_CHUNK * FD), FP16,
                         kind="ExternalInput")
    yout = nc.dram_tensor("yout", (2, N_CHUNK * FD), FP32, kind="ExternalOutput")

    with TileContext(nc) as tc, \
         tc.tile_pool(name="const", bufs=1) as cpool, \
         tc.tile_pool(name="state", bufs=2) as spool, \
         tc.tile_pool(name="work", bufs=3) as wpool, \
         tc.tile_pool(name="psum", bufs=8, space="PSUM") as ppool:

        # two input DMAs on parallel DMA devices: weights+at0 via HWDGE
        # (SP queue), at1..3 via SWDGE (gpsimd queue); fp16 over the wire
        wa = cpool.tile([128, WCOL + FD], FP16, tag="wa")
        nc.sync.dma_start(out=wa[:], in_=ain[:, 0:WCOL + FD])
        a3 = cpool.tile([128, 3 * FD], FP16, tag="a3")
        nc.gpsimd.dma_start(out=a3[:], in_=ain[:, WCOL + FD:])
        wallr = cpool.tile([128, WCOL], FP32R, tag="wallr")
        nc.vector.tensor_copy(wallr[:], wa[:, 0:WCOL])
        w1h = wa[:, 0:128]             # fp16 W1 for the first iteration
        w1r = wallr[:, 0:128]
        w2br = wallr[:, 128:256]
        m2r = wallr[:, 256:384]
        onesr = wallr[:, 384:386]
        ats = [wa[:, WCOL:WCOL + FD]] + \
              [a3[:, i * FD:(i + 1) * FD] for i in range(3)]

        for _rep in range(repeat):
            af = list(ats)              # AF_0 = AT (G_0 = 1; fp32, used once)
            af_ap = [a for a in af]     # APs directly
            us = [None] * N_CHUNK
            gs = [None] * N_CHUNK
            banks = [None] * N_CHUNK

            def emit_halfstep(c, h):
                n = h // 2 + 1          # iteration number, 1-based
                if h % 2 == 0:
                    # S-side: ps = W1 af ; u = 1/(1+S) on ACT
                    # one PSUM bank per chunk-iteration: mm1 -> half 0,
                    # mm2 -> half 1 (PSUM tiles are bank-granular)
                    bank = ppool.tile([128, 2 * FD], FP32, tag="ps")
                    banks[c] = bank
                    ps = bank[:, 0:FD]
                    # iteration 1 runs in fp32 (af = AT straight from DMA);
                    # later iterations in fp32r (1 cyc/row)
                    rhs0 = af[c] if n == 1 else af[c][:]
                    nc.tensor.matmul(out=ps, lhsT=(w1h if n == 1 else w1r),
                                     rhs=rhs0, start=True, stop=True)
                    u_n = spool.tile([128, FD], FP32R, tag=f"u{c}")
                    _act_recip(nc, u_n[:], ps)
                    us[c] = u_n
                else:
                    # T-side: ps2 = wg*W2 u ; G update ; af = AT*G
                    ps2 = banks[c][:, FD:2 * FD]
                    nc.tensor.matmul(out=ps2, lhsT=w2br,
                                     rhs=us[c][:], start=True, stop=True)
                    g_n = spool.tile([128, FD], FP32, tag=f"g{c}")
                    if n == 1:
                        # G_1 = 1/(1+T): ps2 = wg*T, so scale by 1/wg
                        _act_recip(nc, g_n[:], ps2, bias=1.0, scale=1.0 / WG)
                    elif n <= E_G:
                        # exact SOR: ACT emits R = wg/(1+T) directly
                        # (1/(ps2*s + b) with s=1/wg^2, b=1/wg on ps2=wg*T),
                        # then G = (1-wg)*G + R in one stt.
                        r_n = wpool.tile([128, FD], FP32, tag=f"r{c}")
                        _act_recip(nc, r_n[:], ps2,
                                   bias=1.0 / WG, scale=1.0 / (WG * WG))
                        nc.vector.scalar_tensor_tensor(
                            out=g_n[:], in0=gs[c][:], scalar=float(1.0 - WG),
                            in1=r_n[:], op0=mybir.AluOpType.mult,
                            op1=mybir.AluOpType.add)
                    else:
                        # fused Newton+SOR: G = ((1+wg)-(ps2+wg)*G)*G
                        newton_sor(nc.vector, g_n[:], ps2, gs[c][:],
                                   WG, 1.0 + WG)
                    gs[c] = g_n
                    af_n = spool.tile([128, FD], FP32R, tag=f"af{c}")
                    # mul on DVE right behind the G custom: same in-order
                    # queue, no cross-engine semaphore hop
                    nc.vector.tensor_mul(af_n[:], ats[c], gs[c][:])
                    af[c] = af_n

            # chunk c runs c half-steps behind chunk 0: four phases in
            # flight so every engine queue always holds ready work.
            H = 2 * N_ITERS
            for t in range(H + N_CHUNK):
                for c in range(N_CHUNK):
                    h = t - c
                    if 0 <= h < H:
                        emit_halfstep(c, h)

            # readout: V = M2 af; h = V*u (u from the last S-side, half a
            # step stale -- validated numerically); Y = ones^T h.
            # h packed pairwise -> one ones-matmul per pair.
            hpair0 = wpool.tile([128, 2 * FD], FP32R, tag="hp0")
            hpair1 = wpool.tile([128, 2 * FD], FP32R, tag="hp1")
            hpair = [hpair0, hpair1]
            ysall = wpool.tile([128, 4 * FD], FP32, tag="ysall")
            for c in range(N_CHUNK):
                bank = ppool.tile([128, 2 * FD], FP32, tag="ps")
                pp = bank[:, 0:FD]
                nc.tensor.matmul(out=pp, lhsT=m2r, rhs=af[c][:],
                                 start=True, stop=True)
                hslot = hpair[c // 2][:, (c % 2) * FD:(c % 2 + 1) * FD]
                nc.vector.tensor_mul(hslot, pp, us[c][:])
                if c % 2 == 1:
                    p = c // 2
                    ybank = ppool.tile([128, 2 * FD], FP32, tag="ps")
                    nc.tensor.matmul(out=ybank[0:2, :], lhsT=onesr,
                                     rhs=hpair[p][:], start=True, stop=True)
                    nc.scalar.copy(ysall[0:2, p * 2 * FD:(p + 1) * 2 * FD],
                                   ybank[0:2, :])
            nc.sync.dma_start(out=yout[:, :], in_=ysall[0:2, :])

    nc.finalize()
    return nc


def _get_module(repeat=1):
    key = f"nc{repeat}"
    if key not in _CACHE:
        _CACHE[key] = _build_module(repeat)
    return _CACHE[key]


def kernel(AT, K_raw, BT_raw, W_raw, b_raw, _run_kw=None, _repeat=1):
    AT = np.asarray(AT, dtype=np.float32)
    K = np.clip(np.exp(np.asarray(K_raw, np.float32)), 0.0, 1000.0).astype(np.float32)
    BT = np.clip(np.exp(np.asarray(BT_raw, np.float32)), 0.0, 1000.0).astype(np.float32)
    Wc = np.clip(np.asarray(W_raw, np.float32), -10.0, 10.0).reshape(NA, NB)
    b0 = np.clip(np.asarray(b_raw, np.float32), -10.0, 10.0)[0]

    w2 = np.ascontiguousarray((K * BT[None, :]).T)     # (nB,nA) lhsT: T = w2^T u
    M = K * Wc * BT[None, :]                           # (nA,nB) lhsT: V = M^T af

    def blk(a):
        z = np.zeros((128, 128), np.float32)
        z[0:64, 0:64] = a
        z[64:128, 64:128] = a
        return z

    ones2 = np.zeros((128, 2), np.float32)
    ones2[0:64, 0] = 1.0
    ones2[64:128, 1] = 1.0
    wts = np.concatenate([blk(K), blk(WG * w2), blk(M), ones2], axis=1)

    att = np.ascontiguousarray(AT.T)                   # (64, 16384)
    HB = B_CORE // 2                                   # 1024 cols per stream
    in_maps = []
    for c in range(N_CORES):
        chunk = att[:, c * B_CORE:(c + 1) * B_CORE]    # (64, 2048)
        stacked = np.concatenate([chunk[:, :HB], chunk[:, HB:]], axis=0)
        ain = np.ascontiguousarray(
            np.concatenate([wts, stacked], axis=1).astype(np.float16))
        in_maps.append({"ain": ain})

    nc = _get_module(_repeat)
    res = run_bass_kernel_spmd(nc, in_maps, core_ids=list(range(N_CORES)),
                               **(_run_kw or {}))
    out = np.empty((B,), np.float32)
    for co in range(N_CORES):
        y = res.results[co]["yout"]                    # (2, 1024)
        base = co * B_CORE
        out[base:base + HB] = y[0]
        out[base + HB:base + B_CORE] = y[1]
    if _run_kw is not None:
        _CACHE["last_result"] = res
    return out + b0
